# revision 1
# baseline (speedup 1.0000x reference)
"""Trainium2 Bass kernel for the decoupled sparse-attention layer.

Sharding: 8 cores = 2 batch x 4 GQA head-groups. Core i handles batch
b=i//4 and query heads [4g..4g+4) with KV head g, g=i%4. Each core
computes a partial output y_partial = attn_heads @ Wo_rows(group); the
host sums the 4 group partials per batch element.

Per-core layouts (all "transposed": feature dim on partitions):
  xT      [2048, 4096]  input activations, d-major (host pre-transposes)
  W_all   [2048, 384]   fused projection weights, output cols:
            [0:128)   q_sem  4 heads x 32, scaled by exp(ls_h)/sqrt(32)
            [128:192) q_geo first halves (x1), 4 heads x 16, scaled
            [192:256) q_geo second halves (x2), 4 heads x 16, scaled
            [256:288) k_sem 32
            [288:304) k_geo x1 16
            [304:320) k_geo x2 16
            [320:384) v 64
  Keys: 1152 padded slots = [48 mem-blocks | 80 pad | 1024 local].
Matmuls run in float32r (tf32-like, 1 cyc/row at N>=256).
"""

import numpy as np

B, T, D = 2, 4096, 2048
H, HKV, DS, DG, DV = 16, 4, 32, 32, 64
MB, LW = 64, 1024
REMOTE = T - LW            # 3072
NBLK = REMOTE // MB        # 48
NKEY = NBLK + LW           # 1072
KPAD = 128 + LW            # 1152 padded key slots (blocks padded to 128)
NKT = KPAD // 128          # 9 key tiles
TC = 512                   # t-chunk size
NC_CHUNKS = T // TC        # 8
ROPE_BASE = 10000.0
MASK_BIAS = 80.0

_PROG = None


def _active_tiles(c):
    """Key tiles (tile_idx, nrows) visible to query chunk c, plus which
    tiles/rows need masking. Block k fully visible to all q>=512c iff
    k < 8c; partial blocks are [8c, 8c+8). Local tile t (pos 3072+128(t-1)
    ..+127) is fully visible iff its last pos <= 512c."""
    tiles = [(0, 8 * (c + 1))] if c <= 5 else [(0, NBLK)]
    if c >= 6:
        nloc = (c - 5) * TC  # locals visible: 512 for c=6, 1024 for c=7
        for t in range(1, 1 + nloc // 128):
            tiles.append((t, 128))
    # masking: (tile, row_lo, row_hi) slices that need the mask path
    masked = []
    if c <= 5:
        masked.append((0, 8 * c, 8 * (c + 1)))
    else:
        # local tiles whose max pos > 512c need masking
        for t, n in tiles[1:]:
            maxpos = REMOTE + t * 128 - 1
            if maxpos > 512 * c:
                masked.append((t, 0, 128))
    return tiles, masked


def _build_program():
    from contextlib import ExitStack
    import concourse.bass as bass
    import concourse.bacc as bacc
    import concourse.tile as tile
    from concourse import mybir

    f32 = mybir.dt.float32
    f32r = mybir.dt.float32r
    Alu = mybir.AluOpType
    Act = mybir.ActivationFunctionType

    nc = bacc.Bacc()
    xT = nc.declare_dram_parameter("xT", [D, T], f32r, isOutput=False)
    W_all = nc.declare_dram_parameter("W_all", [D, 384], f32r, isOutput=False)
    Wo = nc.declare_dram_parameter("Wo", [256, D], f32r, isOutput=False)
    cosq = nc.declare_dram_parameter("cosq", [64, T], f32, isOutput=False)
    sinq = nc.declare_dram_parameter("sinq", [64, T], f32, isOutput=False)
    kpos = nc.declare_dram_parameter("kpos", [KPAD], f32, isOutput=False)
    qpos = nc.declare_dram_parameter("qpos", [T], f32, isOutput=False)
    ident = nc.declare_dram_parameter("ident", [64, 64], f32, isOutput=False)
    y = nc.declare_dram_parameter("y", [T, D], f32, isOutput=True)

    r = lambda ap: ap  # operands are natively f32r

    with tile.TileContext(nc) as tc, ExitStack() as ctx:
        persist = ctx.enter_context(tc.tile_pool(name="persist", bufs=1))
        xpool = ctx.enter_context(tc.tile_pool(name="x", bufs=20))
        cspool = ctx.enter_context(tc.tile_pool(name="cs", bufs=2))
        tmp = ctx.enter_context(tc.tile_pool(name="tmp", bufs=1))
        epool = ctx.enter_context(tc.tile_pool(name="e", bufs=3))
        ypool = ctx.enter_context(tc.tile_pool(name="y", bufs=2))
        mpool = ctx.enter_context(tc.tile_pool(name="m", bufs=4))
        npool = ctx.enter_context(tc.tile_pool(name="n", bufs=2))
        ps_proj = ctx.enter_context(tc.tile_pool(name="psp", bufs=1, space="PSUM"))
        ps_sc = ctx.enter_context(tc.tile_pool(name="pssc", bufs=2, space="PSUM"))
        ps_out = ctx.enter_context(tc.tile_pool(name="psout", bufs=2, space="PSUM"))
        ps_pv = ctx.enter_context(tc.tile_pool(name="pspv", bufs=1, space="PSUM"))

        # ---- persistent SBUF tensors ----
        wall_sb = persist.tile([128, 16, 384], f32r)   # 16 d-chunks
        for kk in range(16):
            nc.sync.dma_start(out=wall_sb[:, kk, :], in_=W_all[kk * 128:(kk + 1) * 128, :])
        wo_sb = persist.tile([128, 2, D], f32r)        # rows 0:128 / 128:256
        for j in range(2):
            nc.sync.dma_start(out=wo_sb[:, j, :], in_=Wo[j * 128:(j + 1) * 128, :])
        ident_sb = persist.tile([64, 64], f32)
        nc.sync.dma_start(out=ident_sb, in_=ident[:, :])
        kpos_sb = persist.tile([128, NKT], f32)
        nc.sync.dma_start(
            out=kpos_sb,
            in_=bass.AP(tensor=kpos, offset=0, ap=[[1, 128], [128, NKT]]))
        aT01 = persist.tile([128, T], f32r)   # attn out (h dv-major), heads 0,1
        aT23 = persist.tile([128, T], f32r)
        kTd = persist.tile([128, KPAD], f32r)  # pooled keys, duplicated halves
        kT = kTd[0:64, :]                     # build target (lower half)
        vT = persist.tile([64, KPAD], f32)   # pooled values (dv on partitions)
        v2 = persist.tile([128, NKT, 65], f32r)  # [key, dv | ones]
        zsrc = persist.tile([128, 80], f32)
        nc.vector.memset(zsrc, 0.0)
        onesrc = persist.tile([128, 1], f32)
        nc.vector.memset(onesrc, 1.0)
        nc.vector.tensor_copy(out=kT[:, NBLK:128], in_=zsrc[0:64, :])
        nc.vector.memset(vT[:, NBLK:128], 0.0)
        nc.vector.tensor_copy(out=v2[:, 0, :], in_=zsrc[:, 0:65])
        nc.vector.tensor_copy(out=v2[0:NBLK, 0, 64:65], in_=onesrc[0:NBLK, :])
        for t in range(1, NKT):
            nc.vector.tensor_copy(out=v2[:, t, 64:65], in_=onesrc)
        negb = persist.tile([128, 1], f32)
        nc.vector.memset(negb, -MASK_BIAS)

        aTs = [aT01, aT01, aT23, aT23]
        qpool = ctx.enter_context(tc.tile_pool(name="q", bufs=2))

        for c in range(NC_CHUNKS):
            lo = c * TC
            sl = slice(lo, lo + TC)
            # ---- load x chunk + tables ----
            xt = []
            for kk in range(16):
                t_ = xpool.tile([128, TC], f32r, tag="xt")
                nc.sync.dma_start(out=t_, in_=xT[kk * 128:(kk + 1) * 128, sl])
                xt.append(t_)
            cos_t = cspool.tile([64, TC], f32, tag="cos")
            sin_t = cspool.tile([64, TC], f32, tag="sin")
            nc.sync.dma_start(out=cos_t, in_=cosq[:, sl])
            nc.sync.dma_start(out=sin_t, in_=sinq[:, sl])
            qpos_t = cspool.tile([128, TC], f32, tag="qp")
            nc.sync.dma_start(
                out=qpos_t,
                in_=bass.AP(tensor=qpos, offset=lo, ap=[[0, 128], [1, TC]]))

            # ---- projection: psA=q_sem, psB=q_geo(x1|x2), psC=[k|v] ----
            psA = ps_proj.tile([128, TC], f32, tag="psA")
            psB = ps_proj.tile([128, TC], f32, tag="psB")
            psC = ps_proj.tile([128, TC], f32, tag="psC")
            for kk in range(16):
                st, sp = kk == 0, kk == 15
                nc.tensor.matmul(out=psA, lhsT=r(wall_sb[:, kk, 0:128]),
                                 rhs=r(xt[kk]), start=st, stop=sp)
                nc.tensor.matmul(out=psB, lhsT=r(wall_sb[:, kk, 128:256]),
                                 rhs=r(xt[kk]), start=st, stop=sp)
                nc.tensor.matmul(out=psC, lhsT=r(wall_sb[:, kk, 256:384]),
                                 rhs=r(xt[kk]), start=st, stop=sp)

            # ---- assemble q (sem copies + RoPE) ----
            q01 = qpool.tile([128, TC], f32r, tag="q01")
            q23 = qpool.tile([128, TC], f32r, tag="q23")
            qTs = [q01, q01, q23, q23]
            for h in range(4):
                dst = qTs[h]
                base = (h % 2) * 64
                nc.scalar.copy(out=dst[base:base + 32, :],
                               in_=psA[h * 32:(h + 1) * 32, :])
            p1 = tmp.tile([64, TC], f32, tag="p1")
            p2 = tmp.tile([64, TC], f32, tag="p2")
            x1p = tmp.tile([64, TC], f32r, tag="x1p")
            x2p = tmp.tile([64, TC], f32r, tag="x2p")
            nc.vector.tensor_mul(p1, psB[0:64, :], cos_t)
            nc.vector.tensor_mul(p2, psB[64:128, :], sin_t)
            nc.vector.tensor_sub(x1p, p1, p2)
            nc.vector.tensor_mul(p1, psB[64:128, :], cos_t)
            nc.vector.tensor_mul(p2, psB[0:64, :], sin_t)
            nc.vector.tensor_add(x2p, p1, p2)
            for h in range(4):
                dst = qTs[h]
                base = (h % 2) * 64
                hs = slice(h * 16, (h + 1) * 16)
                nc.sync.dma_start(out=dst[base + 32:base + 48, :], in_=x1p[hs, :])
                nc.sync.dma_start(out=dst[base + 48:base + 64, :], in_=x2p[hs, :])

            # ---- assemble k (RoPE) into ktmp, pool k/v ----
            ktmp = tmp.tile([64, TC], f32, tag="ktmp")
            nc.scalar.copy(out=ktmp[0:32, :], in_=psC[0:32, :])
            kg = tmp.tile([32, TC], f32, tag="kg")     # [kx1;kx2] at base 0
            nc.scalar.copy(out=kg, in_=psC[32:64, :])
            kg2 = tmp.tile([16, TC], f32, tag="kg2")   # kx2 alone at base 0
            nc.sync.dma_start(out=kg2, in_=kg[16:32, :])
            k1 = tmp.tile([16, TC], f32, tag="k1")
            k2 = tmp.tile([16, TC], f32, tag="k2")
            k3 = tmp.tile([16, TC], f32, tag="k3")
            nc.vector.tensor_mul(k1, kg[0:16, :], cos_t[0:16, :])
            nc.vector.tensor_mul(k2, kg2, sin_t[0:16, :])
            nc.vector.tensor_sub(k3, k1, k2)
            nc.sync.dma_start(out=ktmp[32:48, :], in_=k3)
            nc.vector.tensor_mul(k1, kg2, cos_t[0:16, :])
            nc.vector.tensor_mul(k2, kg[0:16, :], sin_t[0:16, :])
            nc.vector.tensor_add(k3, k1, k2)
            nc.sync.dma_start(out=ktmp[48:64, :], in_=k3)

            if c <= 5:
                bs = slice(c * 8, (c + 1) * 8)
                with nc.allow_low_precision(reason="fp32r pooled keys"):
                    nc.vector.tensor_reduce(out=kT[:, bs],
                                            in_=ktmp.rearrange("p (n w) -> p n w", w=MB),
                                            axis=mybir.AxisListType.X, op=Alu.add)
                nc.vector.tensor_scalar_mul(kT[:, bs], kT[:, bs], 1.0 / MB)
                nc.vector.tensor_reduce(out=vT[:, bs],
                                        in_=psC[64:128, :].rearrange("p (n w) -> p n w", w=MB),
                                        axis=mybir.AxisListType.X, op=Alu.add)
                nc.vector.tensor_scalar_mul(vT[:, bs], vT[:, bs], 1.0 / MB)
                # incremental V transpose for the block tile
                pv = ps_pv.tile([128, 64], f32, tag="pvt")
                nc.tensor.transpose(out=pv[0:8, :], in_=vT[:, bs], identity=ident_sb)
                vt8 = tmp.tile([8, 64], f32r, tag="vt8")
                nc.scalar.copy(out=vt8, in_=pv[0:8, :])
                nc.sync.dma_start(out=v2[bs, 0, 0:64], in_=vt8)
                nc.sync.dma_start(out=kTd[64:128, bs], in_=kT[:, bs])
            else:
                loff = 128 + (c - 6) * TC
                nc.scalar.copy(out=kT[:, loff:loff + TC], in_=ktmp)
                nc.scalar.copy(out=kTd[64:128, loff:loff + TC], in_=ktmp)
                nc.scalar.copy(out=vT[:, loff:loff + TC], in_=psC[64:128, :])
                for i in range(4):
                    ks = slice(loff + i * 128, loff + (i + 1) * 128)
                    pv = ps_pv.tile([128, 64], f32, tag="pvt")
                    nc.tensor.transpose(out=pv, in_=vT[:, ks], identity=ident_sb)
                    nc.scalar.copy(out=v2[:, 1 + (c - 6) * 4 + i, 0:64], in_=pv)

            # ---- attention for this q chunk ----
            tiles, masked = _active_tiles(c)
            mdict = {}
            ntile = dict(tiles)
            for (mt, rlo, rhi) in masked:
                n = ntile[mt]
                m_sb = mpool.tile([128, TC], f32, tag="mask")
                nc.vector.tensor_scalar(
                    out=m_sb[0:n, :], in0=qpos_t[0:n, :],
                    scalar1=kpos_sb[0:n, mt:mt + 1], scalar2=None,
                    op0=Alu.is_ge)
                mdict[mt] = m_sb
            for h in range(4):
                qb = (h % 2) * 64
                qm = r(qTs[h][qb:qb + 64, :])
                outp = ps_out.tile([65, TC], f32, tag="outp")
                for ti, (kt, n) in enumerate(tiles):
                    sc = ps_sc.tile([128, TC], f32, tag="sc")
                    nc.tensor.matmul(out=sc[0:n, :],
                                     lhsT=r(kTd[qb:qb + 64, kt * 128:kt * 128 + n]),
                                     rhs=qm, start=True, stop=True)
                    if kt in mdict:
                        m_sb = mdict[kt]
                        nc.vector.scalar_tensor_tensor(
                            out=sc[0:n, :], in0=sc[0:n, :],
                            scalar=MASK_BIAS, in1=m_sb[0:n, :],
                            op0=Alu.add, op1=Alu.mult)
                        e_sb = epool.tile([128, TC], f32r, tag="e")
                        nc.scalar.activation(out=e_sb[0:n, :], in_=sc[0:n, :],
                                             func=Act.Exp, bias=negb[0:n, :])
                    else:
                        e_sb = epool.tile([128, TC], f32r, tag="e")
                        nc.scalar.activation(out=e_sb[0:n, :], in_=sc[0:n, :],
                                             func=Act.Exp)
                    nc.tensor.matmul(out=outp, lhsT=r(v2[0:n, kt, :]),
                                     rhs=r(e_sb[0:n, :]),
                                     start=(ti == 0), stop=(ti == len(tiles) - 1))
                r1 = npool.tile([1, TC], f32, tag="r1")
                nc.vector.reciprocal(out=r1, in_=outp[64:65, :])
                rb = npool.tile([64, TC], f32, tag="rb")
                nc.gpsimd.partition_broadcast(out_ap=rb, in_ap=r1)
                base = (h % 2) * 64
                nc.vector.tensor_mul(aTs[h][base:base + 64, sl], outp[0:64, :], rb)

        # ---- uniform rows q in [0, 63): probs = 1/NKEY over all keys ----
        vsum = persist.tile([64, 1], f32)
        nc.vector.tensor_reduce(out=vsum, in_=vT, axis=mybir.AxisListType.X,
                                op=mybir.AluOpType.add)
        nc.vector.tensor_scalar_mul(vsum, vsum, 1.0 / float(NKEY))
        for dst in (aT01, aT23):
            for base in (0, 64):
                nc.vector.tensor_copy(out=dst[base:base + 64, 0:63],
                                      in_=vsum.broadcast_to([64, 63]))

        # ---- output projection ----
        for tt in range(T // 128):
            tsl = slice(tt * 128, (tt + 1) * 128)
            for nn in range(4):
                nsl = slice(nn * 512, (nn + 1) * 512)
                yp = ps_sc.tile([128, 512], f32, tag="sc")
                nc.tensor.matmul(out=yp, lhsT=r(aT01[:, tsl]),
                                 rhs=r(wo_sb[:, 0, nsl]), start=True, stop=False)
                nc.tensor.matmul(out=yp, lhsT=r(aT23[:, tsl]),
                                 rhs=r(wo_sb[:, 1, nsl]), start=False, stop=True)
                y_sb = ypool.tile([128, 512], f32, tag="ysb")
                nc.any.tensor_copy(out=y_sb, in_=yp)
                nc.sync.dma_start(out=y[tsl, nsl], in_=y_sb)
    nc.finalize()
    return nc


def _host_inputs(x, Wq_sem, Wk_sem, Wq_geo, Wk_geo, Wv, Wo, logit_scale):
    """Build the 8 per-core input maps."""
    pos = np.arange(T, dtype=np.float64)
    inv = 1.0 / (ROPE_BASE ** (np.arange(0, DG, 2, dtype=np.float64) / DG))
    ang = pos[:, None] * inv[None, :]              # [T, 16]
    cosq = np.tile(np.cos(ang).T.astype(np.float32), (4, 1))  # [64, T]
    sinq = np.tile(np.sin(ang).T.astype(np.float32), (4, 1))
    kpos = np.full(KPAD, 1e9, dtype=np.float32)
    kpos[:NBLK] = np.arange(NBLK) * MB + (MB - 1)
    kpos[128:] = np.arange(REMOTE, T)
    qpos = np.arange(T, dtype=np.float32)
    ident = np.eye(64, dtype=np.float32)
    xTs = [np.ascontiguousarray(x[b].T) for b in range(B)]

    scale = np.exp(logit_scale.astype(np.float64)).astype(np.float32)
    in_maps = []
    for core in range(8):
        b, g = core // 4, core % 4
        W = np.empty((D, 384), dtype=np.float32)
        for h in range(4):
            gh = 4 * g + h
            s = scale[gh] / np.sqrt(np.float32(DS))
            W[:, h * 32:(h + 1) * 32] = Wq_sem[:, gh * DS:(gh + 1) * DS] * s
            W[:, 128 + h * 16:128 + (h + 1) * 16] = Wq_geo[:, gh * DG:gh * DG + 16] * s
            W[:, 192 + h * 16:192 + (h + 1) * 16] = Wq_geo[:, gh * DG + 16:(gh + 1) * DG] * s
        W[:, 256:288] = Wk_sem[:, g * DS:(g + 1) * DS]
        W[:, 288:304] = Wk_geo[:, g * DG:g * DG + 16]
        W[:, 304:320] = Wk_geo[:, g * DG + 16:(g + 1) * DG]
        W[:, 320:384] = Wv[:, g * DV:(g + 1) * DV]
        in_maps.append({
            "xT": xTs[b],
            "W_all": W,
            "Wo": np.ascontiguousarray(Wo[g * 256:(g + 1) * 256, :]),
            "cosq": cosq, "sinq": sinq, "kpos": kpos, "qpos": qpos,
            "ident": ident,
        })
    return in_maps


def kernel(x, Wq_sem, Wk_sem, Wq_geo, Wk_geo, Wv, Wo, logit_scale, _trace=False):
    global _PROG
    import sys
    if "/opt/trn_rl_repo" not in sys.path:
        sys.path.insert(0, "/opt/trn_rl_repo")
    from concourse.bass_utils import run_bass_kernel_spmd

    x = np.asarray(x, dtype=np.float32)
    in_maps = _host_inputs(np.asarray(x, np.float32),
                           np.asarray(Wq_sem, np.float32),
                           np.asarray(Wk_sem, np.float32),
                           np.asarray(Wq_geo, np.float32),
                           np.asarray(Wk_geo, np.float32),
                           np.asarray(Wv, np.float32),
                           np.asarray(Wo, np.float32),
                           np.asarray(logit_scale, np.float32))
    if _PROG is None:
        _PROG = _build_program()
    res = run_bass_kernel_spmd(_PROG, in_maps, list(range(8)), trace=_trace)
    outs = [res.results[i]["y"] for i in range(8)]
    out = np.empty((B, T, D), dtype=np.float32)
    for b in range(B):
        out[b] = outs[4 * b] + outs[4 * b + 1] + outs[4 * b + 2] + outs[4 * b + 3]
    if _trace:
        return out, res
    return out



# revision 5
# speedup vs baseline: 2.2360x; 2.2360x over previous
"""Trainium2 Bass kernel for the decoupled sparse-attention layer.

Sharding: 8 cores = 2 batch x 4 GQA head-groups. Core i handles batch
b=i//4 and query heads [4g..4g+4) with KV head g, g=i%4. Each core
computes a partial output y_partial = attn_heads @ Wo_rows(group); the
host sums the 4 group partials per batch element.

v2: bf16 datapath (x/W/Wo/k/v/q/e/aT/y), per-chunk inlined output
projection + y DMA (no serial tail), paired score matmuls in disjoint
PE row groups (concurrent), reciprocal_approx_fast for softmax
normalization, mask applied as bf16 multiply on exp output.

Per-core layouts (feature dim on partitions):
  xT      [2048, 4096] bf16 input activations, d-major
  W_all   [2048, 384]  bf16 fused projection weights, output cols:
            [0:128)   q_sem  4 heads x 32, scaled by exp(ls_h)/sqrt(32)
            [128:192) q_geo x1 halves, 4 heads x 16, scaled
            [192:256) q_geo x2 halves, 4 heads x 16, scaled
            [256:288) k_sem 32
            [288:304) k_geo x1 16
            [304:320) k_geo x2 16
            [320:384) v 64
  Keys: 1152 padded slots = [48 mem-blocks | 80 pad | 1024 local].
"""

import numpy as np

B, T, D = 2, 4096, 2048
H, HKV, DS, DG, DV = 16, 4, 32, 32, 64
MB, LW = 64, 1024
REMOTE = T - LW            # 3072
NBLK = REMOTE // MB        # 48
NKEY = NBLK + LW           # 1072
KPAD = 128 + LW            # 1152 padded key slots (blocks padded to 128)
NKT = KPAD // 128          # 9 key tiles
TC = 512                   # t-chunk size
NC_CHUNKS = T // TC        # 8
ROPE_BASE = 10000.0

_PROG = None


def _active_tiles(c):
    """Key tiles (tile_idx, nrows) visible to query chunk c, plus which
    tiles need the mask path. Block k fully visible to all q>=512c iff
    k < 8c; partial blocks are [8c, 8c+8). Local tile t (pos 3072+128(t-1)
    ..+127) is fully visible iff its last pos <= 512c."""
    tiles = [(0, 8 * (c + 1))] if c <= 5 else [(0, NBLK)]
    if c >= 6:
        nloc = (c - 5) * TC
        for t in range(1, 1 + nloc // 128):
            tiles.append((t, 128))
    masked = []
    if c <= 5:
        masked.append(0)
    else:
        for t, n in tiles[1:]:
            maxpos = REMOTE + t * 128 - 1
            if maxpos > 512 * c:
                masked.append(t)
    return tiles, masked


def _build_program():
    from contextlib import ExitStack
    import concourse.bass as bass
    import concourse.bacc as bacc
    import concourse.tile as tile
    from concourse import mybir

    f32 = mybir.dt.float32
    bf16 = mybir.dt.bfloat16
    Alu = mybir.AluOpType
    Act = mybir.ActivationFunctionType

    nc = bacc.Bacc()
    xT = nc.declare_dram_parameter("xT", [D, T], bf16, isOutput=False)
    W_all = nc.declare_dram_parameter("W_all", [D, 384], bf16, isOutput=False)
    Wo = nc.declare_dram_parameter("Wo", [256, D], bf16, isOutput=False)
    cs_c = nc.declare_dram_parameter("cs_c", [128, T], f32, isOutput=False)
    cs_s = nc.declare_dram_parameter("cs_s", [128, T], f32, isOutput=False)
    kq_t = nc.declare_dram_parameter("kq_t", [32, T], f32, isOutput=False)
    kpos = nc.declare_dram_parameter("kpos", [KPAD], f32, isOutput=False)
    qpos = nc.declare_dram_parameter("qpos", [T], f32, isOutput=False)
    ident = nc.declare_dram_parameter("ident", [64, 64], bf16, isOutput=False)
    y = nc.declare_dram_parameter("y", [T, D], bf16, isOutput=True)

    with tile.TileContext(nc) as tc, ExitStack() as ctx, \
            nc.allow_low_precision(reason="bf16 datapath, rel-err budget 2e-2"):
        persist = ctx.enter_context(tc.tile_pool(name="persist", bufs=1))
        xpool = ctx.enter_context(tc.tile_pool(name="x", bufs=3))
        cspool = ctx.enter_context(tc.tile_pool(name="cs", bufs=2))
        tmp = ctx.enter_context(tc.tile_pool(name="tmp", bufs=2))
        epool = ctx.enter_context(tc.tile_pool(name="e", bufs=4))
        ypool = ctx.enter_context(tc.tile_pool(name="y", bufs=2))
        mpool = ctx.enter_context(tc.tile_pool(name="m", bufs=4))
        npool = ctx.enter_context(tc.tile_pool(name="n", bufs=2))
        qpool = ctx.enter_context(tc.tile_pool(name="q", bufs=2))
        ps_proj = ctx.enter_context(tc.tile_pool(name="psp", bufs=1, space="PSUM"))
        ps_sc = ctx.enter_context(tc.tile_pool(name="pssc", bufs=2, space="PSUM"))
        ps_out = ctx.enter_context(tc.tile_pool(name="psout", bufs=2, space="PSUM"))
        ps_pv = ctx.enter_context(tc.tile_pool(name="pspv", bufs=1, space="PSUM"))

        # ---- persistent SBUF tensors ----
        wall_sb = persist.tile([128, 16, 384], bf16)
        nc.sync.dma_start(
            out=wall_sb,
            in_=bass.AP(tensor=W_all, offset=0,
                        ap=[[384, 128], [128 * 384, 16], [1, 384]]))
        wo_sb = persist.tile([128, 2, D], bf16)
        nc.sync.dma_start(
            out=wo_sb,
            in_=bass.AP(tensor=Wo, offset=0,
                        ap=[[D, 128], [128 * D, 2], [1, D]]))
        ident_sb = persist.tile([64, 64], bf16)
        nc.sync.dma_start(out=ident_sb, in_=ident[:, :])
        kpos_sb = persist.tile([128, NKT], f32)
        nc.sync.dma_start(
            out=kpos_sb,
            in_=bass.AP(tensor=kpos, offset=0, ap=[[1, 128], [128, NKT]]))
        aT01 = persist.tile([128, T], bf16)   # attn out heads 0,1
        aT23 = persist.tile([128, T], bf16)   # heads 2,3
        kTd = persist.tile([128, KPAD], bf16)  # keys, duplicated halves
        kT = kTd[0:64, :]
        vT = persist.tile([64, KPAD], bf16)    # pooled values (dv on parts)
        v2 = persist.tile([128, NKT, 65], bf16)  # [key, dv | ones]
        zsrc = persist.tile([128, 80], bf16)
        nc.vector.memset(zsrc, 0.0)
        onesrc = persist.tile([128, 1], bf16)
        nc.vector.memset(onesrc, 1.0)
        nc.vector.tensor_copy(out=kT[:, NBLK:128], in_=zsrc[0:64, :])
        nc.vector.memset(vT[:, NBLK:128], 0.0)
        nc.vector.tensor_copy(out=v2[:, 0, :], in_=zsrc[:, 0:65])
        nc.vector.tensor_copy(out=v2[0:NBLK, 0, 64:65], in_=onesrc[0:NBLK, :])
        for t in range(1, NKT):
            nc.vector.tensor_copy(out=v2[:, t, 64:65], in_=onesrc)

        aTs = [aT01, aT23]

        for c in range(NC_CHUNKS):
            lo = c * TC
            sl = slice(lo, lo + TC)
            # ---- load x chunk + tables ----
            xt = xpool.tile([128, 16, TC], bf16, tag="xt")
            nc.sync.dma_start(
                out=xt,
                in_=bass.AP(tensor=xT, offset=lo,
                            ap=[[T, 128], [128 * T, 16], [1, TC]]))
            csc = cspool.tile([128, TC], f32, tag="csc")
            css = cspool.tile([128, TC], f32, tag="css")
            nc.sync.dma_start(out=csc, in_=cs_c[:, sl])
            nc.sync.dma_start(out=css, in_=cs_s[:, sl])
            kqt = cspool.tile([32, TC], f32, tag="kqt")
            nc.sync.dma_start(out=kqt, in_=kq_t[:, sl])
            qpos_t = cspool.tile([128, TC], f32, tag="qp")
            nc.sync.dma_start(
                out=qpos_t,
                in_=bass.AP(tensor=qpos, offset=lo, ap=[[0, 128], [1, TC]]))

            # ---- projection: psA=q_sem, psB=q_geo(x1|x2), psC=[k|v] ----
            psA = ps_proj.tile([128, TC], f32, tag="psA")
            psB = ps_proj.tile([128, TC], f32, tag="psB")
            psC = ps_proj.tile([128, TC], f32, tag="psC")
            for kk in range(16):
                st, sp = kk == 0, kk == 15
                nc.tensor.matmul(out=psA, lhsT=wall_sb[:, kk, 0:128],
                                 rhs=xt[:, kk, :], start=st, stop=sp)
                nc.tensor.matmul(out=psB, lhsT=wall_sb[:, kk, 128:256],
                                 rhs=xt[:, kk, :], start=st, stop=sp)
                nc.tensor.matmul(out=psC, lhsT=wall_sb[:, kk, 256:384],
                                 rhs=xt[:, kk, :], start=st, stop=sp)

            # ---- assemble q: sem copies + RoPE ----
            q01 = qpool.tile([128, TC], bf16, tag="q01")
            q23 = qpool.tile([128, TC], bf16, tag="q23")
            nc.scalar.copy(out=q01[0:32, :], in_=psA[0:32, :])
            nc.vector.tensor_copy(out=q01[64:96, :], in_=psA[32:64, :])
            nc.scalar.copy(out=q23[0:32, :], in_=psA[64:96, :])
            nc.vector.tensor_copy(out=q23[64:96, :], in_=psA[96:128, :])
            # RoPE on q_geo: psB = [x1(64); x2(64)]
            #   P = psB*cos(tiled), S = psB*sin(tiled)
            #   Rq[0:64] = x1c - x2s = P[0:64] - S[64:128]
            #   Rq[64:128] = x2c + x1s = P[64:128] + S[0:64]
            pP = tmp.tile([128, TC], f32, tag="pP")
            nc.vector.tensor_mul(pP, psB, csc)
            # psB <- psB * sin, in place (PSUM) so the combines below mix
            # SBUF+PSUM operands (two-SBUF inputs must share base partition)
            nc.vector.tensor_mul(psB, psB, css)
            Rq = tmp.tile([128, TC], bf16, tag="Rq")
            nc.vector.tensor_sub(Rq[0:64, :], pP[0:64, :], psB[64:128, :])
            nc.vector.tensor_add(Rq[64:128, :], pP[64:128, :], psB[0:64, :])
            qTs = [q01, q01, q23, q23]
            for h in range(4):
                dst = qTs[h]
                base = (h % 2) * 64
                hs = slice(h * 16, (h + 1) * 16)
                hs2 = slice(64 + h * 16, 64 + (h + 1) * 16)
                nc.sync.dma_start(out=dst[base + 32:base + 48, :], in_=Rq[hs, :])
                nc.sync.dma_start(out=dst[base + 48:base + 64, :], in_=Rq[hs2, :])

            # ---- assemble k (RoPE via swap trick), pool k/v ----
            # kg = [kx1; kx2] (32 parts); kgswap = [kx2; kx1]
            kg = tmp.tile([32, TC], f32, tag="kg")
            nc.scalar.copy(out=kg, in_=psC[32:64, :])
            kgs = tmp.tile([32, TC], f32, tag="kgs")
            nc.sync.dma_start(out=kgs[0:16, :], in_=kg[16:32, :])
            nc.sync.dma_start(out=kgs[16:32, :], in_=kg[0:16, :])
            # Pk = kg*[c;c]; Qk = kgswap*[-s;s]; Rk = Pk+Qk = [kx1'; kx2']
            pk = tmp.tile([32, TC], f32, tag="pk")
            qk = tmp.tile([32, TC], f32, tag="qk")
            nc.vector.tensor_mul(pk, kg, csc[0:32, :])
            nc.vector.tensor_mul(qk, kgs, kqt)

            if c <= 5:
                bs = slice(c * 8, (c + 1) * 8)
                ktmp = tmp.tile([64, TC], f32, tag="ktmp")
                nc.scalar.copy(out=ktmp[0:32, :], in_=psC[0:32, :])
                nc.vector.tensor_add(ktmp[32:64, :], pk, qk)
                nc.vector.tensor_reduce(
                    out=kT[:, bs],
                    in_=ktmp.rearrange("p (n w) -> p n w", w=MB),
                    axis=mybir.AxisListType.X, op=Alu.add)
                nc.vector.tensor_scalar_mul(kT[:, bs], kT[:, bs], 1.0 / MB)
                nc.vector.tensor_reduce(
                    out=vT[:, bs],
                    in_=psC[64:128, :].rearrange("p (n w) -> p n w", w=MB),
                    axis=mybir.AxisListType.X, op=Alu.add)
                nc.vector.tensor_scalar_mul(vT[:, bs], vT[:, bs], 1.0 / MB)
                pv = ps_pv.tile([128, 64], bf16, tag="pvt")
                nc.tensor.transpose(out=pv[0:8, :], in_=vT[:, bs], identity=ident_sb)
                vt8 = tmp.tile([8, 64], bf16, tag="vt8")
                nc.scalar.copy(out=vt8, in_=pv[0:8, :])
                nc.sync.dma_start(out=v2[bs, 0, 0:64], in_=vt8)
                nc.sync.dma_start(out=kTd[64:128, bs], in_=kT[:, bs])
            else:
                loff = 128 + (c - 6) * TC
                lsl = slice(loff, loff + TC)
                nc.scalar.copy(out=kTd[0:32, lsl], in_=psC[0:32, :])
                nc.vector.tensor_add(kTd[32:64, lsl], pk, qk)
                nc.scalar.copy(out=kTd[64:128, lsl], in_=kTd[0:64, lsl])
                nc.scalar.copy(out=vT[:, lsl], in_=psC[64:128, :])
                for i in range(4):
                    ks = slice(loff + i * 128, loff + (i + 1) * 128)
                    pv = ps_pv.tile([128, 64], bf16, tag="pvt")
                    nc.tensor.transpose(out=pv, in_=vT[:, ks], identity=ident_sb)
                    nc.scalar.copy(out=v2[:, 1 + (c - 6) * 4 + i, 0:64], in_=pv)

            # ---- attention for this q chunk ----
            tiles, masked = _active_tiles(c)
            mdict = {}
            ntile = dict(tiles)
            for mt in masked:
                n = ntile[mt]
                m_sb = mpool.tile([128, TC], bf16, tag="mask")
                nc.vector.tensor_scalar(
                    out=m_sb[0:n, :], in0=qpos_t[0:n, :],
                    scalar1=kpos_sb[0:n, mt:mt + 1], scalar2=None,
                    op0=Alu.is_ge)
                mdict[mt] = m_sb
            for pi, (qt, aT) in enumerate([(q01, aT01), (q23, aT23)]):
                outp_a = ps_out.tile([65, TC], f32, tag="outp")
                outp_b = ps_out.tile([65, TC], f32, tag="outp")
                nlast = len(tiles) - 1
                for ti, (kt, n) in enumerate(tiles):
                    ksl = slice(kt * 128, kt * 128 + n)
                    sc_a = ps_sc.tile([128, TC], f32, tag="sc")
                    sc_b = ps_sc.tile([128, TC], f32, tag="sc")
                    nc.tensor.matmul(out=sc_a[0:n, :], lhsT=kTd[0:64, ksl],
                                     rhs=qt[0:64, :], start=True, stop=True)
                    nc.tensor.matmul(out=sc_b[0:n, :], lhsT=kTd[64:128, ksl],
                                     rhs=qt[64:128, :], start=True, stop=True)
                    e_a = epool.tile([128, TC], bf16, tag="e")
                    e_b = epool.tile([128, TC], bf16, tag="e")
                    nc.scalar.activation(out=e_a[0:n, :], in_=sc_a[0:n, :],
                                         func=Act.Exp)
                    nc.scalar.activation(out=e_b[0:n, :], in_=sc_b[0:n, :],
                                         func=Act.Exp)
                    if kt in mdict:
                        m_sb = mdict[kt]
                        nc.vector.tensor_mul(e_a[0:n, :], e_a[0:n, :], m_sb[0:n, :])
                        nc.vector.tensor_mul(e_b[0:n, :], e_b[0:n, :], m_sb[0:n, :])
                    nc.tensor.matmul(out=outp_a, lhsT=v2[0:n, kt, :],
                                     rhs=e_a[0:n, :],
                                     start=(ti == 0), stop=(ti == nlast))
                    nc.tensor.matmul(out=outp_b, lhsT=v2[0:n, kt, :],
                                     rhs=e_b[0:n, :],
                                     start=(ti == 0), stop=(ti == nlast))
                for hb, outp in ((0, outp_a), (64, outp_b)):
                    # custom-DVE recip misreads PSUM operands; stage the
                    # denominator row through SBUF first
                    d1 = npool.tile([1, TC], f32, tag="d1")
                    nc.scalar.copy(out=d1, in_=outp[64:65, :])
                    r1 = npool.tile([1, TC], f32, tag="r1")
                    nc.vector.reciprocal_approx_fast(out=r1, in_=d1)
                    rb = npool.tile([64, TC], f32, tag="rb")
                    nc.gpsimd.partition_broadcast(out_ap=rb, in_ap=r1)
                    nc.vector.tensor_mul(aT[hb:hb + 64, sl], outp[0:64, :], rb)

            # ---- inline output projection for this chunk (defer tt=0) ----
            for tt in range(4 * c, 4 * c + 4):
                if tt == 0:
                    continue
                tsl = slice(tt * 128, (tt + 1) * 128)
                ysb = ypool.tile([128, D], bf16, tag="ysb")
                for nn in range(4):
                    nsl = slice(nn * 512, (nn + 1) * 512)
                    yp = ps_sc.tile([128, TC], f32, tag="sc")
                    nc.tensor.matmul(out=yp, lhsT=aT01[:, tsl],
                                     rhs=wo_sb[:, 0, nsl], start=True, stop=False)
                    nc.tensor.matmul(out=yp, lhsT=aT23[:, tsl],
                                     rhs=wo_sb[:, 1, nsl], start=False, stop=True)
                    nc.any.tensor_copy(out=ysb[:, nsl], in_=yp)
                nc.sync.dma_start(out=y[tsl, :], in_=ysb)

        # ---- uniform rows q in [0, 63): probs = 1/NKEY over all keys ----
        vsum = persist.tile([64, 1], f32)
        nc.vector.tensor_reduce(out=vsum, in_=vT, axis=mybir.AxisListType.X,
                                op=Alu.add)
        nc.vector.tensor_scalar_mul(vsum, vsum, 1.0 / float(NKEY))
        for dst in (aT01, aT23):
            for base in (0, 64):
                nc.vector.tensor_copy(out=dst[base:base + 64, 0:63],
                                      in_=vsum.broadcast_to([64, 63]))
        # deferred out-proj for rows 0:128
        ysb = ypool.tile([128, D], bf16, tag="ysb")
        for nn in range(4):
            nsl = slice(nn * 512, (nn + 1) * 512)
            yp = ps_sc.tile([128, TC], f32, tag="sc")
            nc.tensor.matmul(out=yp, lhsT=aT01[:, 0:128],
                             rhs=wo_sb[:, 0, nsl], start=True, stop=False)
            nc.tensor.matmul(out=yp, lhsT=aT23[:, 0:128],
                             rhs=wo_sb[:, 1, nsl], start=False, stop=True)
            nc.any.tensor_copy(out=ysb[:, nsl], in_=yp)
        nc.sync.dma_start(out=y[0:128, :], in_=ysb)
    nc.finalize()
    return nc


def _host_inputs(x, Wq_sem, Wk_sem, Wq_geo, Wk_geo, Wv, Wo, logit_scale):
    """Build the 8 per-core input maps."""
    import ml_dtypes
    bf16 = ml_dtypes.bfloat16
    pos = np.arange(T, dtype=np.float64)
    inv = 1.0 / (ROPE_BASE ** (np.arange(0, DG, 2, dtype=np.float64) / DG))
    ang = pos[:, None] * inv[None, :]                      # [T, 16]
    c16 = np.cos(ang).T.astype(np.float32)                 # [16, T]
    s16 = np.sin(ang).T.astype(np.float32)
    cs_c = np.tile(c16, (8, 1))                            # [128, T]
    cs_s = np.tile(s16, (8, 1))
    kq_t = np.concatenate([-s16, s16], axis=0)             # [32, T]
    kpos = np.full(KPAD, 1e9, dtype=np.float32)
    kpos[:NBLK] = np.arange(NBLK) * MB + (MB - 1)
    kpos[128:] = np.arange(REMOTE, T)
    qpos = np.arange(T, dtype=np.float32)
    ident = np.eye(64, dtype=np.float32).astype(bf16)
    xTs = [np.ascontiguousarray(x[b].T).astype(bf16) for b in range(B)]

    scale = np.exp(logit_scale.astype(np.float64)).astype(np.float32)
    in_maps = []
    for core in range(8):
        b, g = core // 4, core % 4
        W = np.empty((D, 384), dtype=np.float32)
        for h in range(4):
            gh = 4 * g + h
            s = scale[gh] / np.sqrt(np.float32(DS))
            W[:, h * 32:(h + 1) * 32] = Wq_sem[:, gh * DS:(gh + 1) * DS] * s
            W[:, 128 + h * 16:128 + (h + 1) * 16] = Wq_geo[:, gh * DG:gh * DG + 16] * s
            W[:, 192 + h * 16:192 + (h + 1) * 16] = Wq_geo[:, gh * DG + 16:(gh + 1) * DG] * s
        W[:, 256:288] = Wk_sem[:, g * DS:(g + 1) * DS]
        W[:, 288:304] = Wk_geo[:, g * DG:g * DG + 16]
        W[:, 304:320] = Wk_geo[:, g * DG + 16:(g + 1) * DG]
        W[:, 320:384] = Wv[:, g * DV:(g + 1) * DV]
        in_maps.append({
            "xT": xTs[b],
            "W_all": W.astype(bf16),
            "Wo": np.ascontiguousarray(Wo[g * 256:(g + 1) * 256, :]).astype(bf16),
            "cs_c": cs_c, "cs_s": cs_s, "kq_t": kq_t,
            "kpos": kpos, "qpos": qpos, "ident": ident,
        })
    return in_maps


def kernel(x, Wq_sem, Wk_sem, Wq_geo, Wk_geo, Wv, Wo, logit_scale, _trace=False):
    global _PROG
    import sys
    if "/opt/trn_rl_repo" not in sys.path:
        sys.path.insert(0, "/opt/trn_rl_repo")
    from concourse.bass_utils import run_bass_kernel_spmd

    x = np.asarray(x, dtype=np.float32)
    in_maps = _host_inputs(np.asarray(x, np.float32),
                           np.asarray(Wq_sem, np.float32),
                           np.asarray(Wk_sem, np.float32),
                           np.asarray(Wq_geo, np.float32),
                           np.asarray(Wk_geo, np.float32),
                           np.asarray(Wv, np.float32),
                           np.asarray(Wo, np.float32),
                           np.asarray(logit_scale, np.float32))
    if _PROG is None:
        _PROG = _build_program()
    res = run_bass_kernel_spmd(_PROG, in_maps, list(range(8)), trace=_trace)
    outs = [np.asarray(res.results[i]["y"]).astype(np.float32) for i in range(8)]
    out = np.empty((B, T, D), dtype=np.float32)
    for b in range(B):
        out[b] = outs[4 * b] + outs[4 * b + 1] + outs[4 * b + 2] + outs[4 * b + 3]
    if _trace:
        return out, res
    return out


# revision 11
# speedup vs baseline: 2.2746x; 1.0173x over previous
"""Trainium2 Bass kernel for the decoupled sparse-attention layer.

Sharding: 8 cores = 2 batch x 4 GQA head-groups. Core i handles batch
b=i//4 and query heads [4g..4g+4) with KV head g, g=i%4. Each core
computes a partial output y_partial = attn_heads @ Wo_rows(group); the
host sums the 4 group partials per batch element.

v2: bf16 datapath (x/W/Wo/k/v/q/e/aT/y), per-chunk inlined output
projection + y DMA (no serial tail), paired score matmuls in disjoint
PE row groups (concurrent), reciprocal_approx_fast for softmax
normalization, mask applied as bf16 multiply on exp output.

Per-core layouts (feature dim on partitions):
  xT      [2048, 4096] bf16 input activations, d-major
  W_all   [2048, 384]  bf16 fused projection weights, output cols:
            [0:128)   q_sem  4 heads x 32, scaled by exp(ls_h)/sqrt(32)
            [128:192) q_geo x1 halves, 4 heads x 16, scaled
            [192:256) q_geo x2 halves, 4 heads x 16, scaled
            [256:288) k_sem 32
            [288:304) k_geo x1 16
            [304:320) k_geo x2 16
            [320:384) v 64
  Keys: 1152 padded slots = [48 mem-blocks | 80 pad | 1024 local].
"""

import numpy as np

B, T, D = 2, 4096, 2048
H, HKV, DS, DG, DV = 16, 4, 32, 32, 64
MB, LW = 64, 1024
REMOTE = T - LW            # 3072
NBLK = REMOTE // MB        # 48
NKEY = NBLK + LW           # 1072
KPAD = 128 + LW            # 1152 padded key slots (blocks padded to 128)
NKT = KPAD // 128          # 9 key tiles
TC = 512                   # t-chunk size
NC_CHUNKS = T // TC        # 8
ROPE_BASE = 10000.0

_PROG = None


def _active_tiles(c):
    """Key tiles (tile_idx, nrows) visible to query chunk c, plus which
    tiles need the mask path. Block k fully visible to all q>=512c iff
    k < 8c; partial blocks are [8c, 8c+8). Local tile t (pos 3072+128(t-1)
    ..+127) is fully visible iff its last pos <= 512c."""
    tiles = [(0, 8 * (c + 1))] if c <= 5 else [(0, NBLK)]
    if c >= 6:
        nloc = (c - 5) * TC
        for t in range(1, 1 + nloc // 128):
            tiles.append((t, 128))
    masked = []
    if c <= 5:
        masked.append(0)
    else:
        for t, n in tiles[1:]:
            maxpos = REMOTE + t * 128 - 1
            if maxpos > 512 * c:
                masked.append(t)
    return tiles, masked


def _build_program():
    from contextlib import ExitStack
    import concourse.bass as bass
    import concourse.bacc as bacc
    import concourse.tile as tile
    from concourse import mybir

    f32 = mybir.dt.float32
    bf16 = mybir.dt.bfloat16
    Alu = mybir.AluOpType
    Act = mybir.ActivationFunctionType

    nc = bacc.Bacc()
    xT = nc.declare_dram_parameter("xT", [D, T], bf16, isOutput=False)
    W_all = nc.declare_dram_parameter("W_all", [D, 384], bf16, isOutput=False)
    Wo = nc.declare_dram_parameter("Wo", [256, D], bf16, isOutput=False)
    cs_c = nc.declare_dram_parameter("cs_c", [128, T], f32, isOutput=False)
    cs_s = nc.declare_dram_parameter("cs_s", [128, T], f32, isOutput=False)
    kq_t = nc.declare_dram_parameter("kq_t", [32, T], f32, isOutput=False)
    kpos = nc.declare_dram_parameter("kpos", [KPAD], f32, isOutput=False)
    qpos = nc.declare_dram_parameter("qpos", [T], f32, isOutput=False)
    ident = nc.declare_dram_parameter("ident", [64, 64], bf16, isOutput=False)
    y = nc.declare_dram_parameter("y", [T, D], bf16, isOutput=True)

    with tile.TileContext(nc) as tc, ExitStack() as ctx, \
            nc.allow_low_precision(reason="bf16 datapath, rel-err budget 2e-2"):
        persist = ctx.enter_context(tc.tile_pool(name="persist", bufs=1))
        xpool = ctx.enter_context(tc.tile_pool(name="x", bufs=4))
        cspool = ctx.enter_context(tc.tile_pool(name="cs", bufs=2))
        tmp = ctx.enter_context(tc.tile_pool(name="tmp", bufs=2))
        epool = ctx.enter_context(tc.tile_pool(name="e", bufs=6))
        ypool = ctx.enter_context(tc.tile_pool(name="y", bufs=2))
        mpool = ctx.enter_context(tc.tile_pool(name="m", bufs=4))
        npool = ctx.enter_context(tc.tile_pool(name="n", bufs=2))
        qpool = ctx.enter_context(tc.tile_pool(name="q", bufs=2))
        ps_proj = ctx.enter_context(tc.tile_pool(name="psp", bufs=1, space="PSUM"))
        ps_sc = ctx.enter_context(tc.tile_pool(name="pssc", bufs=2, space="PSUM"))
        ps_out = ctx.enter_context(tc.tile_pool(name="psout", bufs=2, space="PSUM"))
        ps_pv = ctx.enter_context(tc.tile_pool(name="pspv", bufs=1, space="PSUM"))

        # ---- persistent SBUF tensors ----
        wall_sb = persist.tile([128, 16, 384], bf16)
        for qq in range(4):
            nc.sync.dma_start(
                out=wall_sb[:, 4 * qq:4 * qq + 4, :],
                in_=bass.AP(tensor=W_all, offset=qq * 4 * 128 * 384,
                            ap=[[384, 128], [128 * 384, 4], [1, 384]]))
        wo_sb = persist.tile([128, 2, D], bf16)
        nc.sync.dma_start(
            out=wo_sb,
            in_=bass.AP(tensor=Wo, offset=0,
                        ap=[[D, 128], [128 * D, 2], [1, D]]))
        ident_sb = persist.tile([64, 64], bf16)
        nc.sync.dma_start(out=ident_sb, in_=ident[:, :])
        kpos_sb = persist.tile([128, NKT], f32)
        nc.sync.dma_start(
            out=kpos_sb,
            in_=bass.AP(tensor=kpos, offset=0, ap=[[1, 128], [128, NKT]]))
        aT01 = persist.tile([128, T], bf16)   # attn out heads 0,1
        aT23 = persist.tile([128, T], bf16)   # heads 2,3
        kTd = persist.tile([128, KPAD], bf16)  # keys, duplicated halves
        kT = kTd[0:64, :]
        vT = persist.tile([64, KPAD], bf16)    # pooled values (dv on parts)
        v2 = persist.tile([128, NKT, 65], bf16)  # [key, dv | ones]
        zsrc = persist.tile([128, 80], bf16)
        nc.vector.memset(zsrc, 0.0)
        onesrc = persist.tile([128, 1], bf16)
        nc.vector.memset(onesrc, 1.0)
        nc.vector.tensor_copy(out=kT[:, NBLK:128], in_=zsrc[0:64, :])
        nc.vector.memset(vT[:, NBLK:128], 0.0)
        nc.vector.tensor_copy(out=v2[:, 0, :], in_=zsrc[:, 0:65])
        nc.vector.tensor_copy(out=v2[0:NBLK, 0, 64:65], in_=onesrc[0:NBLK, :])
        for t in range(1, NKT):
            nc.vector.tensor_copy(out=v2[:, t, 64:65], in_=onesrc)

        aTs = [aT01, aT23]

        for c in range(NC_CHUNKS):
            lo = c * TC
            sl = slice(lo, lo + TC)
            # ---- load x chunk + tables ----
            xt = xpool.tile([128, 16, TC], bf16, tag="xt")
            for hh in range(2):
                nc.sync.dma_start(
                    out=xt[:, 8 * hh:8 * hh + 8, :],
                    in_=bass.AP(tensor=xT, offset=lo + hh * 8 * 128 * T,
                                ap=[[T, 128], [128 * T, 8], [1, TC]]))
            csc = cspool.tile([128, TC], f32, tag="csc")
            css = cspool.tile([128, TC], f32, tag="css")
            nc.sync.dma_start(out=csc, in_=cs_c[:, sl])
            nc.sync.dma_start(out=css, in_=cs_s[:, sl])
            kqt = cspool.tile([32, TC], f32, tag="kqt")
            nc.sync.dma_start(out=kqt, in_=kq_t[:, sl])
            qpos_t = cspool.tile([128, TC], f32, tag="qp")
            nc.sync.dma_start(
                out=qpos_t,
                in_=bass.AP(tensor=qpos, offset=lo, ap=[[0, 128], [1, TC]]))

            # ---- projection: psA=q_sem, psB=q_geo(x1|x2), psC=[k|v] ----
            psA = ps_proj.tile([128, TC], f32, tag="psA")
            psB = ps_proj.tile([128, TC], f32, tag="psB")
            psC = ps_proj.tile([128, TC], f32, tag="psC")
            for kk in range(16):
                st, sp = kk == 0, kk == 15
                nc.tensor.matmul(out=psA, lhsT=wall_sb[:, kk, 0:128],
                                 rhs=xt[:, kk, :], start=st, stop=sp)
                nc.tensor.matmul(out=psB, lhsT=wall_sb[:, kk, 128:256],
                                 rhs=xt[:, kk, :], start=st, stop=sp)
                nc.tensor.matmul(out=psC, lhsT=wall_sb[:, kk, 256:384],
                                 rhs=xt[:, kk, :], start=st, stop=sp)

            # ---- assemble q: sem copies + RoPE ----
            q01 = qpool.tile([128, TC], bf16, tag="q01")
            q23 = qpool.tile([128, TC], bf16, tag="q23")
            nc.scalar.copy(out=q01[0:32, :], in_=psA[0:32, :])
            nc.vector.tensor_copy(out=q01[64:96, :], in_=psA[32:64, :])
            nc.scalar.copy(out=q23[0:32, :], in_=psA[64:96, :])
            nc.vector.tensor_copy(out=q23[64:96, :], in_=psA[96:128, :])
            # RoPE on q_geo: psB = [x1(64); x2(64)]
            #   P = psB*cos(tiled), S = psB*sin(tiled)
            #   Rq[0:64] = x1c - x2s = P[0:64] - S[64:128]
            #   Rq[64:128] = x2c + x1s = P[64:128] + S[0:64]
            pP = tmp.tile([128, TC], f32, tag="pP")
            nc.vector.tensor_mul(pP, psB, csc)
            # psB <- psB * sin, in place (PSUM) so the combines below mix
            # SBUF+PSUM operands (two-SBUF inputs must share base partition)
            nc.vector.tensor_mul(psB, psB, css)
            Rq = tmp.tile([128, TC], bf16, tag="Rq")
            nc.vector.tensor_sub(Rq[0:64, :], pP[0:64, :], psB[64:128, :])
            nc.vector.tensor_add(Rq[64:128, :], pP[64:128, :], psB[0:64, :])
            qTs = [q01, q01, q23, q23]
            for h in range(4):
                dst = qTs[h]
                base = (h % 2) * 64
                hs = slice(h * 16, (h + 1) * 16)
                hs2 = slice(64 + h * 16, 64 + (h + 1) * 16)
                nc.sync.dma_start(out=dst[base + 32:base + 48, :], in_=Rq[hs, :])
                nc.sync.dma_start(out=dst[base + 48:base + 64, :], in_=Rq[hs2, :])

            # ---- assemble k (RoPE via swap trick), pool k/v ----
            # kg = [kx1; kx2] (32 parts); kgswap = [kx2; kx1]
            kg = tmp.tile([32, TC], f32, tag="kg")
            nc.scalar.copy(out=kg, in_=psC[32:64, :])
            kgs = tmp.tile([32, TC], f32, tag="kgs")
            nc.sync.dma_start(out=kgs[0:16, :], in_=kg[16:32, :])
            nc.sync.dma_start(out=kgs[16:32, :], in_=kg[0:16, :])
            # Pk = kg*[c;c]; Qk = kgswap*[-s;s]; Rk = Pk+Qk = [kx1'; kx2']
            pk = tmp.tile([32, TC], f32, tag="pk")
            qk = tmp.tile([32, TC], f32, tag="qk")
            nc.vector.tensor_mul(pk, kg, csc[0:32, :])
            nc.vector.tensor_mul(qk, kgs, kqt)

            if c <= 5:
                bs = slice(c * 8, (c + 1) * 8)
                ktmp = tmp.tile([64, TC], f32, tag="ktmp")
                nc.scalar.copy(out=ktmp[0:32, :], in_=psC[0:32, :])
                nc.vector.tensor_add(ktmp[32:64, :], pk, qk)
                nc.vector.tensor_reduce(
                    out=kT[:, bs],
                    in_=ktmp.rearrange("p (n w) -> p n w", w=MB),
                    axis=mybir.AxisListType.X, op=Alu.add)
                nc.vector.tensor_scalar_mul(kT[:, bs], kT[:, bs], 1.0 / MB)
                nc.vector.tensor_reduce(
                    out=vT[:, bs],
                    in_=psC[64:128, :].rearrange("p (n w) -> p n w", w=MB),
                    axis=mybir.AxisListType.X, op=Alu.add)
                nc.vector.tensor_scalar_mul(vT[:, bs], vT[:, bs], 1.0 / MB)
                pv = ps_pv.tile([128, 64], bf16, tag="pvt")
                nc.tensor.transpose(out=pv[0:8, :], in_=vT[:, bs], identity=ident_sb)
                vt8 = tmp.tile([8, 64], bf16, tag="vt8")
                nc.scalar.copy(out=vt8, in_=pv[0:8, :])
                nc.sync.dma_start(out=v2[bs, 0, 0:64], in_=vt8)
                nc.sync.dma_start(out=kTd[64:128, bs], in_=kT[:, bs])
            else:
                loff = 128 + (c - 6) * TC
                lsl = slice(loff, loff + TC)
                nc.scalar.copy(out=kTd[0:32, lsl], in_=psC[0:32, :])
                nc.vector.tensor_add(kTd[32:64, lsl], pk, qk)
                nc.scalar.copy(out=kTd[64:128, lsl], in_=kTd[0:64, lsl])
                nc.scalar.copy(out=vT[:, lsl], in_=psC[64:128, :])
                for i in range(4):
                    ks = slice(loff + i * 128, loff + (i + 1) * 128)
                    pv = ps_pv.tile([128, 64], bf16, tag="pvt")
                    nc.tensor.transpose(out=pv, in_=vT[:, ks], identity=ident_sb)
                    nc.scalar.copy(out=v2[:, 1 + (c - 6) * 4 + i, 0:64], in_=pv)

            # ---- attention for this q chunk ----
            tiles, masked = _active_tiles(c)
            mdict = {}
            ntile = dict(tiles)
            for mt in masked:
                n = ntile[mt]
                m_sb = mpool.tile([128, TC], bf16, tag="mask")
                nc.vector.tensor_scalar(
                    out=m_sb[0:n, :], in0=qpos_t[0:n, :],
                    scalar1=kpos_sb[0:n, mt:mt + 1], scalar2=None,
                    op0=Alu.is_ge)
                mdict[mt] = m_sb
            for pi, (qt, aT) in enumerate([(q01, aT01), (q23, aT23)]):
                outp_a = ps_out.tile([65, TC], f32, tag="outp")
                outp_b = ps_out.tile([65, TC], f32, tag="outp")
                nlast = len(tiles) - 1
                for ti, (kt, n) in enumerate(tiles):
                    ksl = slice(kt * 128, kt * 128 + n)
                    sc_a = ps_sc.tile([128, TC], f32, tag="sc")
                    sc_b = ps_sc.tile([128, TC], f32, tag="sc")
                    nc.tensor.matmul(out=sc_a[0:n, :], lhsT=kTd[0:64, ksl],
                                     rhs=qt[0:64, :], start=True, stop=True)
                    nc.tensor.matmul(out=sc_b[0:n, :], lhsT=kTd[64:128, ksl],
                                     rhs=qt[64:128, :], start=True, stop=True)
                    e_a = epool.tile([128, TC], bf16, tag="e")
                    e_b = epool.tile([128, TC], bf16, tag="e")
                    nc.scalar.activation(out=e_a[0:n, :], in_=sc_a[0:n, :],
                                         func=Act.Exp)
                    nc.scalar.activation(out=e_b[0:n, :], in_=sc_b[0:n, :],
                                         func=Act.Exp)
                    if kt in mdict:
                        m_sb = mdict[kt]
                        nc.vector.tensor_mul(e_a[0:n, :], e_a[0:n, :], m_sb[0:n, :])
                        nc.vector.tensor_mul(e_b[0:n, :], e_b[0:n, :], m_sb[0:n, :])
                    nc.tensor.matmul(out=outp_a, lhsT=v2[0:n, kt, :],
                                     rhs=e_a[0:n, :],
                                     start=(ti == 0), stop=(ti == nlast))
                    nc.tensor.matmul(out=outp_b, lhsT=v2[0:n, kt, :],
                                     rhs=e_b[0:n, :],
                                     start=(ti == 0), stop=(ti == nlast))
                for hb, outp in ((0, outp_a), (64, outp_b)):
                    # custom-DVE recip misreads PSUM operands; stage the
                    # denominator row through SBUF first
                    d1 = npool.tile([1, TC], f32, tag="d1")
                    nc.scalar.copy(out=d1, in_=outp[64:65, :])
                    r1 = npool.tile([1, TC], f32, tag="r1")
                    nc.vector.reciprocal_approx_fast(out=r1, in_=d1)
                    rb = npool.tile([64, TC], f32, tag="rb")
                    nc.gpsimd.partition_broadcast(out_ap=rb, in_ap=r1)
                    nc.vector.tensor_mul(aT[hb:hb + 64, sl], outp[0:64, :], rb)

            # ---- inline output projection for this chunk (defer tt=0) ----
            for tt in range(4 * c, 4 * c + 4):
                if tt == 0:
                    continue
                tsl = slice(tt * 128, (tt + 1) * 128)
                ysb = ypool.tile([128, D], bf16, tag="ysb")
                for nn in range(4):
                    nsl = slice(nn * 512, (nn + 1) * 512)
                    yp = ps_sc.tile([128, TC], f32, tag="sc")
                    nc.tensor.matmul(out=yp, lhsT=aT01[:, tsl],
                                     rhs=wo_sb[:, 0, nsl], start=True, stop=False)
                    nc.tensor.matmul(out=yp, lhsT=aT23[:, tsl],
                                     rhs=wo_sb[:, 1, nsl], start=False, stop=True)
                    nc.any.tensor_copy(out=ysb[:, nsl], in_=yp)
                nc.sync.dma_start(out=y[tsl, :], in_=ysb)

        # ---- uniform rows q in [0, 63): probs = 1/NKEY over all keys ----
        vsum = persist.tile([64, 1], f32)
        nc.vector.tensor_reduce(out=vsum, in_=vT, axis=mybir.AxisListType.X,
                                op=Alu.add)
        nc.vector.tensor_scalar_mul(vsum, vsum, 1.0 / float(NKEY))
        for dst in (aT01, aT23):
            for base in (0, 64):
                nc.vector.tensor_copy(out=dst[base:base + 64, 0:63],
                                      in_=vsum.broadcast_to([64, 63]))
        # deferred out-proj for rows 0:128
        ysb = ypool.tile([128, D], bf16, tag="ysb")
        for nn in range(4):
            nsl = slice(nn * 512, (nn + 1) * 512)
            yp = ps_sc.tile([128, TC], f32, tag="sc")
            nc.tensor.matmul(out=yp, lhsT=aT01[:, 0:128],
                             rhs=wo_sb[:, 0, nsl], start=True, stop=False)
            nc.tensor.matmul(out=yp, lhsT=aT23[:, 0:128],
                             rhs=wo_sb[:, 1, nsl], start=False, stop=True)
            nc.any.tensor_copy(out=ysb[:, nsl], in_=yp)
        nc.sync.dma_start(out=y[0:128, :], in_=ysb)
    nc.finalize()
    return nc


def _host_inputs(x, Wq_sem, Wk_sem, Wq_geo, Wk_geo, Wv, Wo, logit_scale):
    """Build the 8 per-core input maps."""
    import ml_dtypes
    bf16 = ml_dtypes.bfloat16
    pos = np.arange(T, dtype=np.float64)
    inv = 1.0 / (ROPE_BASE ** (np.arange(0, DG, 2, dtype=np.float64) / DG))
    ang = pos[:, None] * inv[None, :]                      # [T, 16]
    c16 = np.cos(ang).T.astype(np.float32)                 # [16, T]
    s16 = np.sin(ang).T.astype(np.float32)
    cs_c = np.tile(c16, (8, 1))                            # [128, T]
    cs_s = np.tile(s16, (8, 1))
    kq_t = np.concatenate([-s16, s16], axis=0)             # [32, T]
    kpos = np.full(KPAD, 1e9, dtype=np.float32)
    kpos[:NBLK] = np.arange(NBLK) * MB + (MB - 1)
    kpos[128:] = np.arange(REMOTE, T)
    qpos = np.arange(T, dtype=np.float32)
    ident = np.eye(64, dtype=np.float32).astype(bf16)
    xTs = [np.ascontiguousarray(x[b].T).astype(bf16) for b in range(B)]

    scale = np.exp(logit_scale.astype(np.float64)).astype(np.float32)
    in_maps = []
    for core in range(8):
        b, g = core // 4, core % 4
        W = np.empty((D, 384), dtype=np.float32)
        for h in range(4):
            gh = 4 * g + h
            s = scale[gh] / np.sqrt(np.float32(DS))
            W[:, h * 32:(h + 1) * 32] = Wq_sem[:, gh * DS:(gh + 1) * DS] * s
            W[:, 128 + h * 16:128 + (h + 1) * 16] = Wq_geo[:, gh * DG:gh * DG + 16] * s
            W[:, 192 + h * 16:192 + (h + 1) * 16] = Wq_geo[:, gh * DG + 16:(gh + 1) * DG] * s
        W[:, 256:288] = Wk_sem[:, g * DS:(g + 1) * DS]
        W[:, 288:304] = Wk_geo[:, g * DG:g * DG + 16]
        W[:, 304:320] = Wk_geo[:, g * DG + 16:(g + 1) * DG]
        W[:, 320:384] = Wv[:, g * DV:(g + 1) * DV]
        in_maps.append({
            "xT": xTs[b],
            "W_all": W.astype(bf16),
            "Wo": np.ascontiguousarray(Wo[g * 256:(g + 1) * 256, :]).astype(bf16),
            "cs_c": cs_c, "cs_s": cs_s, "kq_t": kq_t,
            "kpos": kpos, "qpos": qpos, "ident": ident,
        })
    return in_maps


def kernel(x, Wq_sem, Wk_sem, Wq_geo, Wk_geo, Wv, Wo, logit_scale, _trace=False):
    global _PROG
    import sys
    if "/opt/trn_rl_repo" not in sys.path:
        sys.path.insert(0, "/opt/trn_rl_repo")
    from concourse.bass_utils import run_bass_kernel_spmd

    x = np.asarray(x, dtype=np.float32)
    in_maps = _host_inputs(np.asarray(x, np.float32),
                           np.asarray(Wq_sem, np.float32),
                           np.asarray(Wk_sem, np.float32),
                           np.asarray(Wq_geo, np.float32),
                           np.asarray(Wk_geo, np.float32),
                           np.asarray(Wv, np.float32),
                           np.asarray(Wo, np.float32),
                           np.asarray(logit_scale, np.float32))
    if _PROG is None:
        _PROG = _build_program()
    res = run_bass_kernel_spmd(_PROG, in_maps, list(range(8)), trace=_trace)
    outs = [np.asarray(res.results[i]["y"]).astype(np.float32) for i in range(8)]
    out = np.empty((B, T, D), dtype=np.float32)
    for b in range(B):
        out[b] = outs[4 * b] + outs[4 * b + 1] + outs[4 * b + 2] + outs[4 * b + 3]
    if _trace:
        return out, res
    return out
